# revision 20
# baseline (speedup 1.0000x reference)
"""Trainium2 Bass kernel for nn_DecoderWithAttention (Show-Attend-Tell decoder).

Strategy: zero-collective batch parallelism. 8 NeuronCores, 4 batch rows each.
All recurrent weights live SBUF-resident in bf16. Every per-step tensor is kept
in a transposed "column-grouped" layout [128 partitions, (group, b)] so the
128-lane engines stay fully utilized despite the tiny per-core batch:

  X_cg[p, (g, b)] == X_logical[128*g + p, b]

Per step: att2 and gate matmuls contract h against SBUF-resident weights
(weights stationary, out = col-grouped); the additive-attention relu+dot runs
as fused DVE tensor_scalar ops plus a block-column PE matvec; softmax uses a
fused Exp+accumulate on the scalar engine; awe (alpha-weighted encoder sum)
runs as per-sample PE matmuls with block-diagonal alpha columns; the LSTM
gates accumulate emb/awe/h contributions in PSUM; nonlinearities + masked
state blending run on ACT/DVE. The vocab projection (fc) for all (b, t) is
deferred to one batched matmul at the end, with b_fc and the length mask
folded in via a K=1 matmul row.
"""

import numpy as np
import ml_dtypes

BF = ml_dtypes.bfloat16

# Problem dims (hardcoded per contract).
B, PP, E, A, D, EMB, V, L = 32, 196, 2048, 512, 512, 512, 10000, 20
T = L - 1                      # 19 decode steps
NCORES = 8
BL = B // NCORES               # 4 batch rows per core
BT = BL * T                    # 76 (b, t) pairs per core, b-major
EG = E // 128                  # 16 e-groups
GG = 4 * D // 128              # 16 gate-groups
DG = D // 128                  # 4 d-groups
AG = A // 128                  # 4 a-groups
NVT = (V + 511) // 512         # 20 vocab tiles (last = 272)

_CACHE = {}
DEBUG = False


def _build_nc():
    import concourse.bass as bass
    from concourse import bacc
    import concourse.mybir as mybir
    import concourse.tile as tile
    import concourse.tile_sem_assignment as tsa

    # Walrus rejects instructions whose semaphore-wait list exceeds the ISA
    # slot budget (observed: 2 for HWDGE DMA descriptors). Tile's round-robin
    # over 8 HW + 8 SW DMA lanes makes pool-recycled tiles accumulate waits on
    # many distinct lanes. Collapse to one lane of each kind: dependency
    # tracking coarsens (DMAs still run on all queues) and every wait list
    # stays within the ISA budget.
    tsa.NUM_HWDGE_SEMS = 1
    tsa.NUM_SWDGE_GLOBAL_SEMS = 1

    f32 = mybir.dt.float32
    bf16 = mybir.dt.bfloat16
    AF = mybir.ActivationFunctionType
    OP = mybir.AluOpType
    AX = mybir.AxisListType

    nc = bacc.Bacc(None, target_bir_lowering=False)

    def I(name, shape, dt=bf16):
        return nc.dram_tensor(name, shape, dt, kind="ExternalInput")

    # --- DRAM inputs (per-core, host-prepared) ---
    enc_pad = I("enc_pad", [BL, 2, 128, E])          # [b, phalf, p, e], p>=196 zero
    encT = I("encT", [EG, 128, BL * PP])             # [etile, e, (b,p)] b-major
    w_enc = I("w_enc", [EG, 128, A])                 # W_enc_att [e, a] tiled
    b_encT = I("b_encT", [128, AG], f32)
    w_dec = I("w_dec", [DG, 128, A])                 # W_dec_att [d, a] tiled
    b_decT = I("b_decT", [128, AG], f32)
    w_fb = I("w_fb", [DG, 128, E])                   # W_fbeta [d, e] tiled
    bfb_bc = I("bfb_bc", [128, EG, BL], f32)         # b_fbeta col-grouped, b-bcast
    w_ihE = I("w_ihE", [EG, 128, 4 * D])             # W_ih[:, M:].T [e, g] tiled
    w_hh = I("w_hh", [DG, 128, 4 * D])               # W_hh.T [d, g] tiled
    w_ihM = I("w_ihM", [DG, 128, 4 * D])             # W_ih[:, :M].T [d, g] tiled
    biasgT = I("biasgT", [128, GG], f32)             # b_ih + b_hh col-grouped
    embT = I("embT", [DG, 128, BT])                  # emb_seq.T [d, (b,t)]
    w_inith = I("w_inith", [EG, 128, D])
    w_initc = I("w_initc", [EG, 128, D])
    b_inithT = I("b_inithT", [128, DG], f32)
    b_initcT = I("b_initcT", [128, DG], f32)
    wfull_bc = I("wfull_bc", [128, AG, BL, BL])      # W_full block-columns
    onesT_bd = I("onesT_bd", [128, 2, BL, BL])       # 1/196 block-diag (p-valid)
    mask_cg = I("mask_cg", [128, BL, T], f32)        # mask[b,t] bcast over p
    mask_p = I("mask_p", [BL, T], f32)               # mask[b,t], b on partitions
    mask_bt = I("mask_bt", [1, BT])                  # bf16, b-major
    w_fc = I("w_fc", [DG, 128, V])                   # W_fc [d, v] tiled
    b_fc = I("b_fc", [1, V])
    ident = I("ident", [128, 128], f32)

    preds_o = nc.dram_tensor("preds_o", [BT, V], f32, kind="ExternalOutput")
    alph_o = nc.dram_tensor("alph_o", [T, BL, PP], f32, kind="ExternalOutput")
    if DEBUG:
        dbg_h = nc.dram_tensor("dbg_h", [T, 128, DG * BL], f32, kind="ExternalOutput")
        dbg_x = nc.dram_tensor("dbg_x", [6, 128, EG * BL], f32, kind="ExternalOutput")
        dbg_i = nc.dram_tensor("dbg_i", [4, 128, EG * BL], f32, kind="ExternalOutput")
        dbg_w = nc.dram_tensor("dbg_w", [128, 4 * D], bf16, kind="ExternalOutput")
        dbg_hm = nc.dram_tensor("dbg_hm", [128, DG * BL * T], bf16, kind="ExternalOutput")
        dbg_a = nc.dram_tensor("dbg_a", [BL, PP], f32, kind="ExternalOutput")

    with tile.TileContext(nc) as tc:
        with tc.tile_pool(name="state", bufs=1) as st:
            # --- persistent SBUF state ---
            enc_sb = st.tile([128, BL, 2, E], bf16)
            w_dec_sb = st.tile([128, DG, A], bf16)
            w_fb_sb = st.tile([128, DG, E], bf16)
            w_ihE_sb = st.tile([128, EG, 4 * D], bf16)
            w_hh_sb = st.tile([128, DG, 4 * D], bf16)
            att1T = st.tile([128, AG, BL, PP], bf16)
            relu_z = st.tile([128, AG, BL, PP], bf16)
            embproj = st.tile([128, GG, BL, T], f32)
            hist = st.tile([128, DG, BL, T], bf16)
            h2 = st.tile([128, DG, BL], f32)
            c2 = st.tile([128, DG, BL], f32)
            h2b = st.tile([128, DG, BL], bf16)
            meanT = st.tile([128, EG, BL], bf16)
            abd = st.tile([128, 2, BL, BL], bf16)    # alpha block-diag columns
            b_decT_sb = st.tile([128, AG], f32)
            bfb_sb = st.tile([128, EG, BL], f32)
            wfull_sb = st.tile([128, AG, BL, BL], bf16)
            ones_sb = st.tile([128, 2, BL, BL], bf16)
            maskcg_sb = st.tile([128, BL, T], f32)
            maskp_sb = st.tile([BL, T], f32)
            id_sb = st.tile([128, 128], f32)

            # --- phase 0: resident loads ---
            nc.sync.dma_start(enc_sb[:], enc_pad[:].rearrange("b h p e -> p b h e"))
            nc.sync.dma_start(w_dec_sb[:], w_dec[:].rearrange("k p f -> p k f"))
            nc.sync.dma_start(w_fb_sb[:], w_fb[:].rearrange("k p f -> p k f"))
            nc.sync.dma_start(w_ihE_sb[:], w_ihE[:].rearrange("k p f -> p k f"))
            nc.sync.dma_start(w_hh_sb[:], w_hh[:].rearrange("k p f -> p k f"))
            nc.sync.dma_start(b_decT_sb[:], b_decT[:])
            nc.sync.dma_start(bfb_sb[:], bfb_bc[:])
            nc.sync.dma_start(wfull_sb[:], wfull_bc[:])
            nc.sync.dma_start(ones_sb[:], onesT_bd[:])
            nc.sync.dma_start(maskcg_sb[:], mask_cg[:])
            nc.sync.dma_start(maskp_sb[:], mask_p[:])
            nc.sync.dma_start(id_sb[:], ident[:])
            nc.vector.memset(abd[:], 0.0)

            # --- phase 1a: att1 = enc @ W_enc_att (+ b_enc_att) ---
            with (
                tc.tile_pool(name="p1", bufs=3) as p1,
                tc.tile_pool(name="ps1", bufs=1, space="PSUM") as ps1,
            ):
                a1ps = [
                    [ps1.tile([128, 392], f32, tag=f"a1_{ag}_{h}", name=f"a1_{ag}_{h}") for h in range(2)]
                    for ag in range(AG)
                ]
                b_encT_sb = p1.tile([128, AG], f32, tag="bencT")
                nc.sync.dma_start(b_encT_sb[:], b_encT[:])
                we_c = [p1.tile([128, 4, A], bf16, tag=f"we{j}", name=f"we{j}", bufs=1)
                        for j in range(4)]
                eT_c = [p1.tile([128, 4, BL * PP], bf16, tag=f"eT{j}", name=f"eT{j}", bufs=1)
                        for j in range(4)]
                for j in range(4):
                    nc.sync.dma_start(
                        we_c[j][:], w_enc[4 * j:4 * (j + 1)].rearrange("k p f -> p k f"))
                    nc.sync.dma_start(
                        eT_c[j][:], encT[4 * j:4 * (j + 1)].rearrange("k p f -> p k f"))
                for kt in range(EG):
                    we_t = we_c[kt // 4][:, kt % 4]
                    eT_t = eT_c[kt // 4][:, kt % 4]
                    for ag in range(AG):
                        for h in range(2):
                            nc.tensor.matmul(
                                a1ps[ag][h][:],
                                we_t[:, 128 * ag:128 * (ag + 1)],
                                eT_t[:, 392 * h:392 * (h + 1)],
                                start=(kt == 0), stop=(kt == EG - 1),
                            )
                for ag in range(AG):
                    for h in range(2):
                        nc.scalar.activation(
                            att1T[:, ag, 2 * h:2 * h + 2, :], a1ps[ag][h][:],
                            AF.Identity, bias=b_encT_sb[:, ag:ag + 1],
                        )

            # --- phase 1b: mean_enc, h0, c0 ---
            tc.strict_bb_all_engine_barrier()
            with (
                tc.tile_pool(name="p1b", bufs=3) as p1b,
                tc.tile_pool(name="ps1b", bufs=1, space="PSUM") as ps1b,
            ):
                mps = ps1b.tile([128, EG, BL], f32, tag="mean")
                for et in range(EG):
                    first = True
                    for b in range(BL):
                        for h in range(2):
                            nc.tensor.matmul(
                                mps[:, et, :],
                                enc_sb[:, b, h, 128 * et:128 * (et + 1)],
                                ones_sb[:, h, b, :],
                                start=first, stop=(b == BL - 1 and h == 1),
                            )
                            first = False
                nc.vector.tensor_copy(meanT[:], mps[:])

                b_ihT_sb = p1b.tile([128, DG], f32, tag="bih")
                nc.gpsimd.dma_start(b_ihT_sb[:], b_inithT[:])
                b_icT_sb = p1b.tile([128, DG], f32, tag="bic")
                nc.gpsimd.dma_start(b_icT_sb[:], b_initcT[:])
                hps = ps1b.tile([128, DG, BL], f32, tag="h0")
                cps = ps1b.tile([128, DG, BL], f32, tag="c0")
                wh_c = [p1b.tile([128, 4, D], bf16, tag=f"wih{j}", name=f"wih{j}", bufs=1)
                        for j in range(4)]
                wc_c = [p1b.tile([128, 4, D], bf16, tag=f"wic{j}", name=f"wic{j}", bufs=1)
                        for j in range(4)]
                for j in range(4):
                    nc.gpsimd.dma_start(
                        wh_c[j][:], w_inith[4 * j:4 * (j + 1)].rearrange("k p f -> p k f"))
                    nc.gpsimd.dma_start(
                        wc_c[j][:], w_initc[4 * j:4 * (j + 1)].rearrange("k p f -> p k f"))
                if DEBUG:
                    nc.gpsimd.dma_start(dbg_w[:], wh_c[0][:].rearrange("p k f -> p (k f)"))
                # NB: PSUM start=True clears has_written for the WHOLE bank,
                # so accumulation groups sharing a bank must run sequentially,
                # never interleaved (dg outer, kt inner).
                for dg in range(DG):
                    for kt in range(EG):
                        nc.tensor.matmul(
                            hps[:, dg, :],
                            wh_c[kt // 4][:, kt % 4, 128 * dg:128 * (dg + 1)],
                            meanT[:, kt, :],
                            start=(kt == 0), stop=(kt == EG - 1),
                        )
                    for kt in range(EG):
                        nc.tensor.matmul(
                            cps[:, dg, :],
                            wc_c[kt // 4][:, kt % 4, 128 * dg:128 * (dg + 1)],
                            meanT[:, kt, :],
                            start=(kt == 0), stop=(kt == EG - 1),
                        )
                for dg in range(DG):
                    nc.scalar.activation(h2[:, dg, :], hps[:, dg, :], AF.Identity,
                                         bias=b_ihT_sb[:, dg:dg + 1])
                    nc.scalar.activation(c2[:, dg, :], cps[:, dg, :], AF.Identity,
                                         bias=b_icT_sb[:, dg:dg + 1])
                nc.vector.tensor_copy(h2b[:], h2[:])

                # --- phase 1c: embproj = emb_seq @ W_ih[:, :M].T + b_ih + b_hh ---
                biasg_sb = p1b.tile([128, GG], f32, tag="bg")
                nc.gpsimd.dma_start(biasg_sb[:], biasgT[:])
                embT_sb = p1b.tile([128, DG, BT], bf16, tag="embT")
                nc.gpsimd.dma_start(embT_sb[:], embT[:].rearrange("k p f -> p k f"))
                wm = [p1b.tile([128, 4 * D], bf16, tag=f"wm{kt}", name=f"wm{kt}", bufs=1) for kt in range(DG)]
                for kt in range(DG):
                    nc.gpsimd.dma_start(wm[kt][:], w_ihM[kt])
                for gg in range(GG):
                    eps_ = ps1b.tile([128, BT], f32, tag="ep")
                    for kt in range(DG):
                        nc.tensor.matmul(
                            eps_[:], wm[kt][:, 128 * gg:128 * (gg + 1)],
                            embT_sb[:, kt, :],
                            start=(kt == 0), stop=(kt == DG - 1),
                        )
                    nc.scalar.activation(embproj[:, gg, :, :], eps_[:], AF.Identity,
                                         bias=biasg_sb[:, gg:gg + 1])

            if DEBUG:
                with tc.tile_pool(name="dbgi", bufs=1) as dbp:
                    di = dbp.tile([128, EG, BL], f32)
                    nc.vector.memset(di[:], 0.0)
                    nc.vector.tensor_copy(di[:, 0:DG, :], h2[:])
                    nc.vector.tensor_copy(di[:, DG:2*DG, :], c2[:])
                    nc.gpsimd.dma_start(dbg_i[0], di[:].rearrange("p a b -> p (a b)"))
                    dm = dbp.tile([128, EG, BL], f32)
                    nc.vector.tensor_copy(dm[:], meanT[:])
                    nc.gpsimd.dma_start(dbg_i[1], dm[:].rearrange("p a b -> p (a b)"))
                    dep = dbp.tile([128, EG, BL], f32)
                    nc.vector.tensor_copy(dep[:], embproj[:, :, :, 0])
                    nc.gpsimd.dma_start(dbg_i[2], dep[:].rearrange("p a b -> p (a b)"))
                    da1 = dbp.tile([128, EG, BL], f32)
                    nc.vector.memset(da1[:], 0.0)
                    nc.vector.tensor_copy(da1[:, 0:AG, :], att1T[:, :, :, 0])
                    nc.gpsimd.dma_start(dbg_i[3], da1[:].rearrange("p a b -> p (a b)"))

            # --- phase 2: the 19 recurrent steps ---
            tc.strict_bb_all_engine_barrier()
            with (
                tc.tile_pool(name="sp", bufs=2) as sp,
                tc.tile_pool(name="ps2", bufs=1, space="PSUM") as ps2,
            ):
                for t in range(T):
                    # att2 = h @ W_dec_att  -> [128, (ag, b)] psum
                    a2ps = ps2.tile([128, AG, BL], f32, tag="att2")
                    for ag in range(AG):
                        for kt in range(DG):
                            nc.tensor.matmul(
                                a2ps[:, ag, :],
                                w_dec_sb[:, kt, 128 * ag:128 * (ag + 1)],
                                h2b[:, kt, :],
                                start=(kt == 0), stop=(kt == DG - 1),
                            )
                    a2sb = sp.tile([128, AG, BL], f32, tag="a2sb")
                    nc.vector.tensor_tensor(
                        a2sb[:], a2ps[:],
                        b_decT_sb[:, :, None].to_broadcast((128, AG, BL)),
                        OP.add,
                    )
                    # relu_z = relu(att1 + att2)   (fused add+max on DVE)
                    for ag in range(AG):
                        for b in range(BL):
                            nc.vector.tensor_scalar(
                                relu_z[:, ag, b, :], att1T[:, ag, b, :],
                                a2sb[:, ag, b:b + 1], 0.0, OP.add, OP.max,
                            )
                    # e = relu_z @ W_full  -> [4, 196] psum (block-column lhsT)
                    eps = ps2.tile([BL, PP], f32, tag="e")
                    first = True
                    for ag in range(AG):
                        for b in range(BL):
                            nc.tensor.matmul(
                                eps[:], wfull_sb[:, ag, b, :], relu_z[:, ag, b, :],
                                start=first, stop=(ag == AG - 1 and b == BL - 1),
                            )
                            first = False
                    # softmax over p (shift-invariant: b_full skipped)
                    negmx = sp.tile([BL, 1], f32, tag="negmx")
                    nc.vector.tensor_reduce(negmx[:], eps[:], AX.X, OP.max,
                                            negate=True)
                    exp_sb = sp.tile([BL, PP], f32, tag="exp")
                    den = sp.tile([BL, 1], f32, tag="den")
                    nc.scalar.activation(exp_sb[:], eps[:], AF.Exp,
                                         bias=negmx[:], accum_out=den[:])
                    rden = sp.tile([BL, 1], f32, tag="rden")
                    nc.vector.reciprocal(rden[:], den[:])
                    alpha = sp.tile([BL, PP], f32, tag="alpha")
                    nc.vector.tensor_scalar_mul(alpha[:], exp_sb[:], rden[:])
                    # masked alpha -> output
                    rdm = sp.tile([BL, 1], f32, tag="rdm")
                    nc.vector.tensor_tensor(rdm[:], rden[:], maskp_sb[:, t:t + 1],
                                            OP.mult)
                    alpha_o = sp.tile([BL, PP], f32, tag="alpha_o")
                    nc.vector.tensor_scalar_mul(alpha_o[:], exp_sb[:], rdm[:])
                    nc.gpsimd.dma_start(alph_o[t], alpha_o[:])
                    if DEBUG and t == 0:
                        nc.gpsimd.dma_start(dbg_a[:], alpha[:])
                    # transpose alpha -> [196, 4] in psum, then block-diag cols
                    at0 = ps2.tile([128, BL], f32, tag="at0")
                    at1 = ps2.tile([128, BL], f32, tag="at1")
                    nc.tensor.transpose(at0[:], alpha[:, 0:128], id_sb[:BL, :BL])
                    nc.tensor.transpose(at1[:68, :], alpha[:, 128:PP],
                                        id_sb[:BL, :BL])
                    for b in range(BL):
                        nc.vector.tensor_copy(abd[:, 0, b, b:b + 1],
                                              at0[:, b:b + 1])
                        nc.vector.tensor_copy(abd[:68, 1, b, b:b + 1],
                                              at1[:68, b:b + 1])
                    # awe^T[e, b] via block-diag alpha columns
                    awps = ps2.tile([128, EG, BL], f32, tag="awe")
                    for et in range(EG):
                        first = True
                        for b in range(BL):
                            for h in range(2):
                                nc.tensor.matmul(
                                    awps[:, et, :],
                                    enc_sb[:, b, h, 128 * et:128 * (et + 1)],
                                    abd[:, h, b, :],
                                    start=first, stop=(b == BL - 1 and h == 1),
                                )
                                first = False
                    # gate = sigmoid(h @ W_fbeta + b_fbeta)
                    gps = ps2.tile([128, EG, BL], f32, tag="gate")
                    for eg in range(EG):
                        for kt in range(DG):
                            nc.tensor.matmul(
                                gps[:, eg, :],
                                w_fb_sb[:, kt, 128 * eg:128 * (eg + 1)],
                                h2b[:, kt, :],
                                start=(kt == 0), stop=(kt == DG - 1),
                            )
                    gsum = sp.tile([128, EG, BL], f32, tag="gsum")
                    nc.vector.tensor_tensor(gsum[:], gps[:], bfb_sb[:], OP.add)
                    gate = sp.tile([128, EG, BL], f32, tag="gatesb")
                    nc.scalar.activation(gate[:], gsum[:], AF.Sigmoid)
                    # x_E = gate * awe  (bf16 for the gates matmul)
                    xe = sp.tile([128, EG, BL], bf16, tag="xe")
                    nc.vector.tensor_tensor(xe[:], gate[:], awps[:], OP.mult)
                    # gates = x_E @ W_ihE + h @ W_hh  (+ embproj via DVE)
                    gaps = ps2.tile([128, GG, BL], f32, tag="gates")
                    for gg in range(GG):
                        for et in range(EG):
                            nc.tensor.matmul(
                                gaps[:, gg, :],
                                w_ihE_sb[:, et, 128 * gg:128 * (gg + 1)],
                                xe[:, et, :],
                                start=(et == 0), stop=False,
                            )
                        for kt in range(DG):
                            nc.tensor.matmul(
                                gaps[:, gg, :],
                                w_hh_sb[:, kt, 128 * gg:128 * (gg + 1)],
                                h2b[:, kt, :],
                                start=False, stop=(kt == DG - 1),
                            )
                    gg_sum = sp.tile([128, GG, BL], f32, tag="ggsum")
                    nc.vector.tensor_tensor(gg_sum[:], gaps[:],
                                            embproj[:, :, :, t], OP.add)
                    # nonlinearities (i, f, g, o in gate-group order)
                    acts = sp.tile([128, GG, BL], f32, tag="acts")
                    nc.scalar.activation(acts[:, 0:8, :], gg_sum[:, 0:8, :],
                                         AF.Sigmoid)
                    nc.scalar.activation(acts[:, 8:12, :], gg_sum[:, 8:12, :],
                                         AF.Tanh)
                    nc.scalar.activation(acts[:, 12:16, :], gg_sum[:, 12:16, :],
                                         AF.Sigmoid)
                    cn = sp.tile([128, DG, BL], f32, tag="cn")
                    nc.vector.tensor_tensor(cn[:], acts[:, 4:8, :], c2[:], OP.mult)
                    tmp = sp.tile([128, DG, BL], f32, tag="tmp")
                    nc.vector.tensor_tensor(tmp[:], acts[:, 0:4, :],
                                            acts[:, 8:12, :], OP.mult)
                    nc.vector.tensor_tensor(cn[:], cn[:], tmp[:], OP.add)
                    tc_t = sp.tile([128, DG, BL], f32, tag="tc")
                    nc.scalar.activation(tc_t[:], cn[:], AF.Tanh)
                    hn = sp.tile([128, DG, BL], f32, tag="hn")
                    nc.vector.tensor_tensor(hn[:], acts[:, 12:16, :], tc_t[:],
                                            OP.mult)
                    # masked state blending + masked h for fc
                    mk = maskcg_sb[:, None, :, t].to_broadcast((128, DG, BL))
                    dc = sp.tile([128, DG, BL], f32, tag="dc")
                    nc.vector.tensor_tensor(dc[:], cn[:], c2[:], OP.subtract)
                    nc.vector.tensor_tensor(dc[:], dc[:], mk, OP.mult)
                    nc.vector.tensor_tensor(c2[:], c2[:], dc[:], OP.add)
                    dh = sp.tile([128, DG, BL], f32, tag="dh")
                    nc.vector.tensor_tensor(dh[:], hn[:], h2[:], OP.subtract)
                    nc.vector.tensor_tensor(dh[:], dh[:], mk, OP.mult)
                    nc.vector.tensor_tensor(h2[:], h2[:], dh[:], OP.add)
                    nc.vector.tensor_tensor(hist[:, :, :, t], hn[:], mk, OP.mult)
                    nc.vector.tensor_copy(h2b[:], h2[:])
                    if DEBUG:
                        nc.gpsimd.dma_start(dbg_h[t], h2[:].rearrange("p a b -> p (a b)"))
                        if t == 0:
                            dx = sp.tile([128, EG, BL], f32, tag="dbgx")
                            nc.vector.tensor_copy(dx[:], awps[:])
                            nc.gpsimd.dma_start(dbg_x[0], dx[:].rearrange("p a b -> p (a b)"))
                            nc.gpsimd.dma_start(dbg_x[1], gate[:].rearrange("p a b -> p (a b)"))
                            dx2 = sp.tile([128, EG, BL], f32, tag="dbgx2")
                            nc.vector.tensor_copy(dx2[:], gg_sum[:])
                            nc.gpsimd.dma_start(dbg_x[2], dx2[:].rearrange("p a b -> p (a b)"))
                            dx3 = sp.tile([128, EG, BL], f32, tag="dbgx3")
                            nc.vector.tensor_copy(dx3[:], acts[:])
                            nc.gpsimd.dma_start(dbg_x[3], dx3[:].rearrange("p a b -> p (a b)"))
                            dx4 = sp.tile([128, EG, BL], f32, tag="dbgx4")
                            nc.vector.memset(dx4[:], 0.0)
                            nc.vector.tensor_copy(dx4[:, 0:DG, :], hn[:])
                            nc.vector.tensor_copy(dx4[:, DG:2*DG, :], cn[:])
                            nc.gpsimd.dma_start(dbg_x[4], dx4[:].rearrange("p a b -> p (a b)"))
                            dx5 = sp.tile([128, EG, BL], f32, tag="dbgx5")
                            nc.vector.tensor_copy(dx5[:], xe[:])
                            nc.gpsimd.dma_start(dbg_x[5], dx5[:].rearrange("p a b -> p (a b)"))

            if DEBUG:
                nc.gpsimd.dma_start(dbg_hm[:], hist[:].rearrange("p a b t -> p (a b t)"))

            # --- phase 3: preds = (h_new @ W_fc + b_fc) * mask ---
            tc.strict_bb_all_engine_barrier()
            with (
                tc.tile_pool(name="fcp", bufs=4) as fcp,
                tc.tile_pool(name="ps3", bufs=2, space="PSUM") as ps3,
            ):
                maskbt_sb = fcp.tile([1, BT], bf16, bufs=1)
                nc.gpsimd.dma_start(maskbt_sb[:], mask_bt[:])
                bfc_sb = fcp.tile([1, V], bf16, bufs=1)
                nc.gpsimd.dma_start(bfc_sb[:], b_fc[:])
                for nt in range(NVT):
                    nw = min(512, V - 512 * nt)
                    fps = ps3.tile([128, 512], f32, tag="fc")
                    wt = fcp.tile([128, DG, 512], bf16, tag="wfc")
                    nc.gpsimd.dma_start(
                        wt[:, :, :nw],
                        w_fc[:, :, 512 * nt:512 * nt + nw].rearrange("k p f -> p k f"))
                    for kt in range(DG):
                        nc.tensor.matmul(
                            fps[:BT, :nw], hist[:, kt, :, :], wt[:, kt, :nw],
                            start=(kt == 0), stop=False,
                        )
                    nc.tensor.matmul(
                        fps[:BT, :nw], maskbt_sb[:],
                        bfc_sb[:, 512 * nt:512 * nt + nw],
                        start=False, stop=True,
                    )
                    ot = fcp.tile([128, 512], f32, tag="ofc")
                    nc.vector.tensor_copy(ot[:BT, :nw], fps[:BT, :nw])
                    nc.gpsimd.dma_start(preds_o[:, 512 * nt:512 * nt + nw], ot[:BT, :nw])

    nc.finalize()
    return nc


def _host_prep(inputs):
    """Sort batch, gather embeddings, tile/transpose weights -> per-core in_maps."""
    f32 = np.float32
    g = {k: np.asarray(v) for k, v in inputs.items()}
    lengths = g["caption_lengths"]
    sort_ind = np.argsort(-lengths, kind="stable")
    enc = g["encoder_out"].reshape(B, PP, E).astype(f32)[sort_ind]
    caps = g["encoded_captions"][sort_ind]
    dec_len = (lengths[sort_ind] - 1).astype(np.int64)
    emb_seq = g["emb"].astype(f32)[caps[:, :T]]          # [B, T, EMB]
    mask = (dec_len[:, None] > np.arange(T)[None, :]).astype(f32)  # [B, T]

    W_ihT = g["W_ih"].astype(f32).T                       # [M+E, 4D]
    biasg = (g["b_ih"] + g["b_hh"]).astype(f32)

    def cg(v, ng):                                        # [ng*128] -> [128, ng]
        return np.ascontiguousarray(v.reshape(ng, 128).T.astype(f32))

    shared = {
        "w_enc": g["W_enc_att"].astype(BF).reshape(EG, 128, A),
        "b_encT": cg(g["b_enc_att"], AG),
        "w_dec": g["W_dec_att"].astype(BF).reshape(DG, 128, A),
        "b_decT": cg(g["b_dec_att"], AG),
        "w_fb": g["W_fbeta"].astype(BF).reshape(DG, 128, E),
        "bfb_bc": np.ascontiguousarray(
            np.broadcast_to(cg(g["b_fbeta"], EG)[:, :, None], (128, EG, BL))),
        "w_ihE": W_ihT[EMB:].astype(BF).reshape(EG, 128, 4 * D),
        "w_hh": g["W_hh"].astype(f32).T.astype(BF).reshape(DG, 128, 4 * D),
        "w_ihM": W_ihT[:EMB].astype(BF).reshape(DG, 128, 4 * D),
        "biasgT": cg(biasg, GG),
        "w_inith": g["W_init_h"].astype(BF).reshape(EG, 128, D),
        "w_initc": g["W_init_c"].astype(BF).reshape(EG, 128, D),
        "b_inithT": cg(g["b_init_h"], DG),
        "b_initcT": cg(g["b_init_c"], DG),
        "w_fc": g["W_fc"].astype(BF).reshape(DG, 128, V),
        "b_fc": g["b_fc"].astype(BF).reshape(1, V),
        "ident": np.eye(128, dtype=f32),
    }
    wf = np.zeros((128, AG, BL, BL), f32)
    for ag in range(AG):
        for b in range(BL):
            wf[:, ag, b, b] = g["W_full"].astype(f32)[128 * ag:128 * (ag + 1), 0]
    shared["wfull_bc"] = wf.astype(BF)
    ob = np.zeros((128, 2, BL, BL), f32)
    for h in range(2):
        n = min(128, PP - 128 * h)
        for b in range(BL):
            ob[:n, h, b, b] = 1.0 / PP
    shared["onesT_bd"] = ob.astype(BF)

    in_maps = []
    for c in range(NCORES):
        sl = slice(BL * c, BL * (c + 1))
        enc_c = enc[sl]                                   # [BL, PP, E]
        pad = np.zeros((BL, 256, E), f32)
        pad[:, :PP] = enc_c
        m_c = mask[sl]                                    # [BL, T]
        im = dict(shared)
        im["enc_pad"] = pad.reshape(BL, 2, 128, E).astype(BF)
        im["encT"] = np.ascontiguousarray(
            enc_c.transpose(2, 0, 1).reshape(EG, 128, BL * PP)).astype(BF)
        im["embT"] = np.ascontiguousarray(
            emb_seq[sl].transpose(2, 0, 1).reshape(DG, 128, BT)).astype(BF)
        im["mask_cg"] = np.ascontiguousarray(
            np.broadcast_to(m_c[None], (128, BL, T)))
        im["mask_p"] = np.ascontiguousarray(m_c)
        im["mask_bt"] = m_c.reshape(1, BT).astype(BF)
        in_maps.append(im)

    meta = {"sort_ind": sort_ind, "caps": caps, "lengths_dtype": lengths.dtype}
    return in_maps, meta


def kernel(**inputs):
    from concourse.bass_utils import run_bass_kernel_spmd

    if "nc" not in _CACHE:
        _CACHE["nc"] = _build_nc()
    nc = _CACHE["nc"]
    in_maps, meta = _host_prep(inputs)
    res = run_bass_kernel_spmd(nc, in_maps, core_ids=list(range(NCORES)))
    _CACHE["last_results"] = res
    predictions = np.concatenate(
        [r["preds_o"].reshape(BL, T, V) for r in res.results], axis=0)
    alphas = np.concatenate(
        [r["alph_o"].transpose(1, 0, 2) for r in res.results], axis=0)
    idx_dt = np.asarray(inputs["caption_lengths"]).dtype
    sort_ind = meta["sort_ind"].astype(idx_dt)
    return predictions, meta["caps"], alphas, sort_ind
